# revision 10
# baseline (speedup 1.0000x reference)
"""Farthest-point-sampling (npoint=2) Bass kernel for Trainium2.

Problem: xyz [1, 64, 3, 262144] fp32 -> indices [64, 2] (int64 on host).
Per batch b:
  idx0 = argmax_n y[n]            (y = coord plane 1)
  c    = (x,y,z)[idx0]
  idx1 = argmax_n ((x-cx)^2 + (y-cy)^2 + (z-cz)^2)
argmax = first occurrence on ties (jnp.argmax semantics).

Sharding: data-parallel over batch; 8 NeuronCores x 8 batches each.

Per-core pipeline (per batch, planes viewed as [128, 2048] fp32):
  - DMA y plane; VectorE Max8 + MaxIndex -> per-partition (max, col)
  - cross-partition argmax finale: PE-transpose per-partition (max,
    N-globalidx) pairs to one partition, then reduce/select tiny ops.
    candidate = N - global_idx so taking the max picks the smallest
    index among equal maxima (first-occurrence tie semantics).
  - centroid gather: idx0 -> Pool register (values_load) -> dynamic-
    offset DMA of [3,1] from DRAM; negate; partition_broadcast -> [128,3]
  - DMA x,z planes; ScalarE Square(v + (-c)) for the three planes
  - s1 = sqx + sqy on GpSimd; VectorE tensor_tensor_reduce fuses
    s2 = s1 + sqz with the per-partition max; MaxIndex for the column.
  - dist finales for all 8 batches are deferred and batched at the end.
GPSIMD ucode: only 'proxy'-library ops (tensor_tensor, partition_broadcast)
plus stock iota/memset/affine_select - loaded once at start.
"""

import numpy as np

import concourse.bacc as bacc
import concourse.bass as bass
import concourse.mybir as mybir
from concourse.masks import make_identity
from concourse.tile import TileContext

B = 64  # full batch
N_CORES = 8
BPC = B // N_CORES  # batches per core
N = 262144
P = 128
COLS = N // P  # 2048
BIGK = float(N)
NEG_INF = -3.0e38

F32 = mybir.dt.float32
U32 = mybir.dt.uint32
I32 = mybir.dt.int32
AX = mybir.AxisListType.X
OP = mybir.AluOpType


def build_nc(add_engine="gpsimd"):
    nc = bacc.Bacc()
    xin = nc.dram_tensor("xyz", [BPC, 3, N], F32, kind="ExternalInput")
    out = nc.dram_tensor("idx", [1, 2 * BPC], I32, kind="ExternalOutput")

    with TileContext(nc) as tc:
        with (
            tc.tile_pool(name="consts", bufs=1) as consts,
            tc.tile_pool(name="big", bufs=2) as big,
            tc.tile_pool(name="small", bufs=3) as small,
            tc.tile_pool(name="acc", bufs=1) as acc,
            tc.tile_pool(name="psum", bufs=2, space="PSUM") as psum,
        ):
            # --- constants (stock gpsimd ucode, then swap to 'proxy') ---
            ident = consts.tile([P, P], F32)
            make_identity(nc, ident)
            # revb[p] = N - p*COLS  (int32 -> f32, exact below 2^24)
            revb_i = consts.tile([P, 1], I32)
            nc.gpsimd.iota(revb_i, pattern=[[0, 1]], base=N, channel_multiplier=-COLS)
            revb_f = consts.tile([P, 1], F32)
            nc.vector.tensor_copy(revb_f, revb_i)
            # plane base offsets {0, N, 2N} for the indirect centroid gather
            pbase = consts.tile([3, 1], I32)
            nc.gpsimd.iota(pbase, pattern=[[0, 1]], base=0, channel_multiplier=N)

            out_f = acc.tile([1, 2 * BPC], F32)   # cols 0..7 idx0, 8..15 idx1
            deferV = acc.tile([P, BPC], F32)      # per-batch dist max  [P,1]
            deferC = acc.tile([P, BPC], F32)      # per-batch N-globalidx

            for b in range(BPC):
                # ---------- phase A: argmax over y ----------
                ty = big.tile([P, COLS], F32, tag="ty")
                nc.sync.dma_start(ty, xin[b, 1].rearrange("(p m) -> p m", p=P))
                m8y = small.tile([P, 8], F32, tag="m8y")
                nc.vector.max(out=m8y, in_=ty)
                i8y = small.tile([P, 8], U32, tag="i8y")
                nc.vector.max_index(i8y, m8y, ty)

                i_f = small.tile([P, 1], F32, tag="i_f")
                nc.vector.tensor_copy(i_f, i8y[:, 0:1])
                candall = small.tile([P, 1], F32, tag="candall")
                nc.vector.tensor_sub(candall, revb_f, i_f)

                pt = psum.tile([1, 2 * P], F32, tag="pt")
                nc.tensor.transpose(pt[0:1, 0:P], m8y[:, 0:1], ident)
                nc.tensor.transpose(pt[0:1, P : 2 * P], candall, ident)
                row = small.tile([1, 2 * P], F32, tag="row")
                nc.vector.tensor_copy(row, pt)
                mx = small.tile([1, 1], F32, tag="mx")
                nc.vector.tensor_reduce(mx, row[0:1, 0:P], axis=AX, op=OP.max)
                cand = small.tile([1, P], F32, tag="cand")
                nc.vector.scalar_tensor_tensor(
                    out=cand,
                    in0=row[0:1, 0:P],
                    scalar=mx[0:1, 0:1],
                    in1=row[0:1, P : 2 * P],
                    op0=OP.is_equal,
                    op1=OP.mult,
                )
                r = small.tile([1, 1], F32, tag="r")
                nc.vector.tensor_reduce(r, cand, axis=AX, op=OP.max)
                idx0f = small.tile([1, 1], F32, tag="idx0f")
                nc.vector.tensor_scalar(
                    out=idx0f, in0=r, scalar1=-1.0, scalar2=BIGK,
                    op0=OP.mult, op1=OP.add,
                )
                nc.scalar.copy(out_f[0:1, b : b + 1], idx0f)

                # ---------- centroid gather + broadcast ----------
                idx0u = small.tile([1, 1], U32, tag="idx0u")
                nc.vector.tensor_copy(idx0u, idx0f)
                idx3 = small.tile([3, 1], U32, tag="idx3")
                nc.gpsimd.partition_broadcast(idx3, idx0u[0:1, :], channels=3)
                offs = small.tile([3, 1], U32, tag="offs")
                # offs[c] = idx0 + b*3N + c*N  (flat index into xin)
                nc.vector.scalar_tensor_tensor(
                    out=offs, in0=idx3, scalar=float(b * 3 * N), in1=pbase,
                    op0=OP.add, op1=OP.add,
                )
                c3 = small.tile([3, 1], F32, tag="c3")
                nc.gpsimd.indirect_dma_start(
                    out=c3,
                    out_offset=None,
                    in_=xin.rearrange("b c n -> (b c n)")[:, None],
                    in_offset=bass.IndirectOffsetOnAxis(ap=offs[0:3, 0:1], axis=0),
                )
                crow = small.tile([1, 3], F32, tag="crow")
                for c in range(3):
                    nc.sync.dma_start(crow[0:1, c : c + 1], c3[c : c + 1, 0:1])
                negrow = small.tile([1, 3], F32, tag="negrow")
                nc.scalar.mul(negrow, crow, -1.0)
                negc = small.tile([P, 3], F32, tag="negc")
                nc.gpsimd.partition_broadcast(negc, negrow[0:1, :], channels=P)

                # ---------- phase B: distances ----------
                tx = big.tile([P, COLS], F32, tag="tx")
                nc.sync.dma_start(tx, xin[b, 0].rearrange("(p m) -> p m", p=P))
                tz = big.tile([P, COLS], F32, tag="tz")
                nc.sync.dma_start(tz, xin[b, 2].rearrange("(p m) -> p m", p=P))

                sqx = big.tile([P, COLS], F32, tag="sqx")
                nc.scalar.activation(
                    sqx, tx, mybir.ActivationFunctionType.Square, bias=negc[:, 0:1]
                )
                sqy = big.tile([P, COLS], F32, tag="sqy")
                nc.scalar.activation(
                    sqy, ty, mybir.ActivationFunctionType.Square, bias=negc[:, 1:2]
                )
                sqz = big.tile([P, COLS], F32, tag="sqz")
                nc.scalar.activation(
                    sqz, tz, mybir.ActivationFunctionType.Square, bias=negc[:, 2:3]
                )

                s1 = big.tile([P, COLS], F32, tag="s1")
                nc.gpsimd.tensor_add(s1, sqx, sqy)
                s2 = big.tile([P, COLS], F32, tag="s2")
                nc.gpsimd.tensor_add(s2, s1, sqz)

                m8d = small.tile([P, 8], F32, tag="m8d")
                nc.vector.max(out=m8d, in_=s2)
                nc.vector.tensor_copy(deferV[:, b : b + 1], m8d[:, 0:1])
                i8d = small.tile([P, 8], U32, tag="i8d")
                nc.vector.max_index(i8d, m8d, s2)
                i_fd = small.tile([P, 1], F32, tag="i_fd")
                nc.vector.tensor_copy(i_fd, i8d[:, 0:1])
                nc.vector.tensor_sub(deferC[:, b : b + 1], revb_f, i_fd)

            # ---------- batched dist finales ----------
            ptv = psum.tile([BPC, 2 * P], F32, tag="ptv")
            nc.tensor.transpose(ptv[0:BPC, 0:P], deferV, ident)
            nc.tensor.transpose(ptv[0:BPC, P : 2 * P], deferC, ident)
            rows = small.tile([BPC, 2 * P], F32, tag="rows")
            nc.vector.tensor_copy(rows, ptv)
            mxs = small.tile([BPC, 1], F32, tag="mxs")
            nc.vector.tensor_reduce(mxs, rows[:, 0:P], axis=AX, op=OP.max)
            cands = small.tile([BPC, P], F32, tag="cands")
            nc.vector.scalar_tensor_tensor(
                out=cands,
                in0=rows[:, 0:P],
                scalar=mxs[:, 0:1],
                in1=rows[:, P : 2 * P],
                op0=OP.is_equal,
                op1=OP.mult,
            )
            rs = small.tile([BPC, 1], F32, tag="rs")
            nc.vector.tensor_reduce(rs, cands, axis=AX, op=OP.max)
            idx1s = small.tile([BPC, 1], F32, tag="idx1s")
            nc.vector.tensor_scalar(
                out=idx1s, in0=rs, scalar1=-1.0, scalar2=BIGK,
                op0=OP.mult, op1=OP.add,
            )
            pti = psum.tile([1, BPC], F32, tag="pti")
            nc.tensor.transpose(pti, idx1s, ident[0:BPC, 0:BPC])
            nc.vector.tensor_copy(out_f[0:1, BPC : 2 * BPC], pti)

            out_i = acc.tile([1, 2 * BPC], I32)
            nc.vector.tensor_copy(out_i, out_f)
            nc.sync.dma_start(out[:, :], out_i[:, :])

    nc.compile()
    return nc


_NC_CACHE = None


def _get_nc():
    global _NC_CACHE
    if _NC_CACHE is None:
        _NC_CACHE = build_nc()
    return _NC_CACHE


def kernel(xyz: np.ndarray) -> np.ndarray:
    from concourse.bass_utils import run_bass_kernel_spmd

    assert xyz.shape == (1, B, 3, N), xyz.shape
    xyz = np.ascontiguousarray(xyz, dtype=np.float32)
    nc = _get_nc()
    in_maps = [
        {"xyz": np.ascontiguousarray(xyz[0, k * BPC : (k + 1) * BPC])}
        for k in range(N_CORES)
    ]
    res = run_bass_kernel_spmd(nc, in_maps, core_ids=list(range(N_CORES)))
    # out layout per core: [1, 16] = [idx0 x8 | idx1 x8]
    outs = [res.results[k]["idx"].reshape(2, BPC).T for k in range(N_CORES)]
    return np.concatenate(outs, axis=0).astype(np.int64)


# revision 12
# speedup vs baseline: 1.6726x; 1.6726x over previous
"""Farthest-point-sampling (npoint=2) Bass kernel for Trainium2.

Problem: xyz [1, 64, 3, 262144] fp32 -> indices [64, 2] (int64 on host).
Per batch b:
  idx0 = argmax_n y[n]            (y = coord plane 1)
  c    = (x,y,z)[idx0]
  idx1 = argmax_n ((x-cx)^2 + (y-cy)^2 + (z-cz)^2)
argmax = first occurrence on ties (jnp.argmax semantics).

Sharding: data-parallel over batch; 8 NeuronCores x 8 batches each.

Per-core structure (planes viewed as [128, 2048] fp32):
  Phase 0 (all 8 batches): DMA y plane; VectorE Max8 + MaxIndex ->
    per-partition (top-8, cols); stash col-0 max and (N - global_idx)
    candidate into defer tiles.
  Y finale (batched): PE-transpose the 8 batches' [128,1] pairs into
    rows, then reduce/select tiny ops produce idx0 per batch.
    candidate = N - global_idx so the max picks the smallest index among
    equal maxima (first-occurrence tie semantics).
  Per batch phase B: PE ones-matmul broadcasts idx0 -> [3,1]; offsets
    stt; indirect-DMA gather of centroid [3,1]; PE transpose + ScalarE
    negate -> [1,3]; PE ones-matmul broadcast -> [128,3] bias tile;
    paired x+z DMA; ScalarE Square(v + (-c)) x3; GpSimd adds
    s1 = sqx+sqy, s2 = s1+sqz; VectorE Max8 + MaxIndex on s2; stash.
  Dist finale (batched): same as Y finale -> idx1 per batch.
All GPSIMD ops are 'standard'-library (iota, tensor_tensor) or DGE —
no mid-kernel ucode library swaps.
"""

import numpy as np

import concourse.bacc as bacc
import concourse.bass as bass
import concourse.mybir as mybir
from concourse.masks import make_identity
from concourse.tile import TileContext

B = 64  # full batch
N_CORES = 8
BPC = B // N_CORES  # batches per core
N = 262144
P = 128
COLS = N // P  # 2048
BIGK = float(N)

F32 = mybir.dt.float32
U32 = mybir.dt.uint32
I32 = mybir.dt.int32
AX = mybir.AxisListType.X
OP = mybir.AluOpType
SQUARE = mybir.ActivationFunctionType.Square


def build_nc():
    nc = bacc.Bacc()
    xin = nc.dram_tensor("xyz", [BPC, 3, N], F32, kind="ExternalInput")
    out = nc.dram_tensor("idx", [1, 2 * BPC], I32, kind="ExternalOutput")

    with TileContext(nc) as tc:
        with (
            tc.tile_pool(name="consts", bufs=1) as consts,
            tc.tile_pool(name="ypool", bufs=BPC) as ypool,
            tc.tile_pool(name="big", bufs=2) as big,
            tc.tile_pool(name="small", bufs=4) as small,
            tc.tile_pool(name="acc", bufs=1) as acc,
            tc.tile_pool(name="psb", bufs=2, space="PSUM") as psb,
            tc.tile_pool(name="psf", bufs=1, space="PSUM") as psf,
        ):
            # ---- constants ----
            ident = consts.tile([P, P], F32)
            make_identity(nc, ident)
            ones = consts.tile([1, P], F32)
            nc.vector.memset(ones, 1.0)
            # revb[p] = N - p*COLS ; pbase[c] = c*N   (exact in f32 < 2^24)
            revb_i = consts.tile([P, 1], I32)
            nc.gpsimd.iota(revb_i, pattern=[[0, 1]], base=N, channel_multiplier=-COLS)
            revb_f = consts.tile([P, 1], F32)
            nc.vector.tensor_copy(revb_f, revb_i)
            pbase = consts.tile([3, 1], I32)
            nc.gpsimd.iota(pbase, pattern=[[0, 1]], base=0, channel_multiplier=N)

            out_i = acc.tile([1, 2 * BPC], I32)  # cols 0..7 idx0, 8..15 idx1
            dYV = acc.tile([P, BPC], F32)
            dYC = acc.tile([P, BPC], F32)
            dDV = acc.tile([P, BPC], F32)
            dDC = acc.tile([P, BPC], F32)

            def batched_finale(dv, dc, out_cols, tagp):
                """dv/dc: [P, BPC] per-batch (per-partition max, N-gidx).
                Returns SBUF [1, BPC] f32 of winning indices; also writes
                them (cast i32) into out_i[:, out_cols]."""
                ptv = psf.tile([BPC, 2 * P], F32, tag="ptv")
                nc.tensor.transpose(ptv[0:BPC, 0:P], dv, ident)
                nc.tensor.transpose(ptv[0:BPC, P : 2 * P], dc, ident)
                rows = small.tile([BPC, 2 * P], F32, tag=f"rows{tagp}")
                nc.vector.tensor_copy(rows, ptv)
                mxs = small.tile([BPC, 1], F32, tag=f"mxs{tagp}")
                nc.vector.tensor_reduce(mxs, rows[:, 0:P], axis=AX, op=OP.max)
                cands = small.tile([BPC, P], F32, tag=f"cands{tagp}")
                nc.vector.scalar_tensor_tensor(
                    out=cands,
                    in0=rows[:, 0:P],
                    scalar=mxs[:, 0:1],
                    in1=rows[:, P : 2 * P],
                    op0=OP.is_equal,
                    op1=OP.mult,
                )
                rs = small.tile([BPC, 1], F32, tag=f"rs{tagp}")
                nc.vector.tensor_reduce(rs, cands, axis=AX, op=OP.max)
                idxs = small.tile([BPC, 1], F32, tag=f"idxs{tagp}")
                nc.vector.tensor_scalar(
                    out=idxs, in0=rs, scalar1=-1.0, scalar2=BIGK,
                    op0=OP.mult, op1=OP.add,
                )
                pti = psf.tile([1, BPC], F32, tag="pti")
                nc.tensor.transpose(pti, idxs, ident[0:BPC, 0:BPC])
                rowi = small.tile([1, BPC], F32, tag=f"rowi{tagp}")
                nc.vector.tensor_copy(rowi, pti)
                nc.scalar.copy(out_i[0:1, out_cols], rowi)
                return rowi

            # ---------- phase 0: y argmax per batch ----------
            tys = []
            for b in range(BPC):
                ty = ypool.tile([P, COLS], F32, tag="ty")
                tys.append(ty)
                nc.sync.dma_start(ty, xin[b, 1].rearrange("(p m) -> p m", p=P))
                m8y = small.tile([P, 8], F32, tag="m8y")
                nc.vector.max(out=m8y, in_=ty)
                i8y = small.tile([P, 8], U32, tag="i8y")
                nc.vector.max_index(i8y, m8y, ty)
                nc.vector.tensor_copy(dYV[:, b : b + 1], m8y[:, 0:1])
                nc.vector.tensor_sub(dYC[:, b : b + 1], revb_f, i8y[:, 0:1])

            idx0row = batched_finale(dYV, dYC, slice(0, BPC), "y")

            # ---------- phase B per batch ----------
            for b in range(BPC):
                # idx0 -> [3,1] via PE ones-matmul; offsets; gather centroid
                p3 = psb.tile([3, 1], F32, tag="p3")
                nc.tensor.matmul(
                    p3, ones[0:1, 0:3], idx0row[0:1, b : b + 1],
                    start=True, stop=True,
                )
                offs = small.tile([3, 1], U32, tag="offs")
                # offs[c] = idx0 + b*3N + c*N (flat index into xin)
                nc.vector.scalar_tensor_tensor(
                    out=offs, in0=p3, scalar=float(b * 3 * N), in1=pbase,
                    op0=OP.add, op1=OP.add,
                )
                c3 = small.tile([3, 1], F32, tag="c3")
                nc.gpsimd.indirect_dma_start(
                    out=c3,
                    out_offset=None,
                    in_=xin.rearrange("b c n -> (b c n)")[:, None],
                    in_offset=bass.IndirectOffsetOnAxis(ap=offs[0:3, 0:1], axis=0),
                )
                # negate + broadcast to [128,3] bias tile via PE
                pc3 = psb.tile([1, 3], F32, tag="pc3")
                nc.tensor.transpose(pc3, c3, ident[0:3, 0:3])
                negrow = small.tile([1, 3], F32, tag="negrow")
                nc.scalar.mul(negrow, pc3, -1.0)
                pnegc = psb.tile([P, 3], F32, tag="pnegc")
                nc.tensor.matmul(pnegc, ones, negrow, start=True, stop=True)
                negc = small.tile([P, 3], F32, tag="negc")
                nc.vector.tensor_copy(negc, pnegc)

                # x and z planes in one strided DMA: [P, 2, COLS]
                txz = big.tile([P, 2, COLS], F32, tag="txz")
                nc.sync.dma_start(
                    txz,
                    xin[b, 0::2].rearrange("c (p m) -> p c m", p=P),
                )
                sqx = big.tile([P, COLS], F32, tag="sqx")
                nc.scalar.activation(sqx, txz[:, 0], SQUARE, bias=negc[:, 0:1])
                sqy = big.tile([P, COLS], F32, tag="sqy")
                nc.scalar.activation(sqy, tys[b], SQUARE, bias=negc[:, 1:2])
                sqz = big.tile([P, COLS], F32, tag="sqz")
                nc.scalar.activation(sqz, txz[:, 1], SQUARE, bias=negc[:, 2:3])

                # s1 = sqx + sqy -> reuse txz[:,0]; s2 = s1 + sqz -> txz[:,1]
                s1 = big.tile([P, COLS], F32, tag="s1")
                nc.gpsimd.tensor_add(s1, sqx, sqy)
                s2 = big.tile([P, COLS], F32, tag="s2")
                nc.gpsimd.tensor_add(s2, s1, sqz)

                m8d = small.tile([P, 8], F32, tag="m8d")
                nc.vector.max(out=m8d, in_=s2)
                i8d = small.tile([P, 8], U32, tag="i8d")
                nc.vector.max_index(i8d, m8d, s2)
                nc.vector.tensor_copy(dDV[:, b : b + 1], m8d[:, 0:1])
                nc.vector.tensor_sub(dDC[:, b : b + 1], revb_f, i8d[:, 0:1])

            batched_finale(dDV, dDC, slice(BPC, 2 * BPC), "d")

            nc.sync.dma_start(out[:, :], out_i[:, :])

    nc.compile()
    return nc


_NC_CACHE = None


def _get_nc():
    global _NC_CACHE
    if _NC_CACHE is None:
        _NC_CACHE = build_nc()
    return _NC_CACHE


def kernel(xyz: np.ndarray) -> np.ndarray:
    from concourse.bass_utils import run_bass_kernel_spmd

    assert xyz.shape == (1, B, 3, N), xyz.shape
    xyz = np.ascontiguousarray(xyz, dtype=np.float32)
    nc = _get_nc()
    in_maps = [
        {"xyz": np.ascontiguousarray(xyz[0, k * BPC : (k + 1) * BPC])}
        for k in range(N_CORES)
    ]
    res = run_bass_kernel_spmd(nc, in_maps, core_ids=list(range(N_CORES)))
    # out layout per core: [1, 16] = [idx0 x8 | idx1 x8]
    outs = [res.results[k]["idx"].reshape(2, BPC).T for k in range(N_CORES)]
    return np.concatenate(outs, axis=0).astype(np.int64)
